# revision 5
# baseline (speedup 1.0000x reference)
"""GaussianImage_Cholesky rasterizer on 8 Trainium2 NeuronCores.

Strategy: pixel-parallel across cores (core i owns image rows 64i..64i+63).
Host culls gaussians per image row (pixels further than ry in y can never
reach alpha >= 1/255, which the reference zeroes), gathers per-row slabs
padded to a multiple of 128, and precomputes per-(row, segment) quadratic
coefficients so the device evaluates

    sigma[g, px] = w0_g*px_s^2 + w1_g*px_s + w2_g        (px_s segment-local)

as a K=3 fp32 matmul. Per row on device:
    4x matmul fp32  : sigma psum[128,512]  (one per 128-px segment)
    ACT Exp         : alpha = exp(-sigma) -> fp16 sbuf
    DVE (fused)     : alpha = (alpha >= 1/255) * alpha
    matmul fp16     : out psum[7,512] = C[128,7].T @ alpha  (rgb,geom,ones)
    DMA             : psum -> dram out[row]
Host reassembles rows, clips, and emits (render, gauss_render, alpha_map,
opac).
"""
import sys
import numpy as np

if "/opt/trn_rl_repo" not in sys.path:
    sys.path.insert(0, "/opt/trn_rl_repo")

N_CORES = 8
H_IMG = 512
W_IMG = 512
ROWS_PER_CORE = H_IMG // N_CORES
NSEG = 4
SEGW = W_IMG // NSEG
SCUT = float(np.log(255.0))
CULL_MARGIN = 0.03
PAD_SIGMA = 100.0

_compiled = {}


def _build(n_chunks, reps=1):
    """Compile the SPMD kernel for a given number of 128-gaussian chunks/row.

    reps>1 wraps the row loop in a hardware For_i that re-executes the whole
    body `reps` times — used only for marginal-cost timing in test.py.
    """
    if (n_chunks, reps) in _compiled:
        return _compiled[(n_chunks, reps)]

    import concourse.tile as tile
    from concourse import bacc, mybir

    nc = bacc.Bacc("TRN2", target_bir_lowering=False, debug=False,
                   num_devices=N_CORES)

    w1 = nc.dram_tensor("w1", [3 * n_chunks, ROWS_PER_CORE * NSEG * 128],
                        mybir.dt.float32, kind="ExternalInput")
    c2 = nc.dram_tensor("c2", [128, ROWS_PER_CORE * n_chunks * 7],
                        mybir.dt.float16, kind="ExternalInput")
    p3 = nc.dram_tensor("p3", [3, W_IMG], mybir.dt.float32,
                        kind="ExternalInput")
    out = nc.dram_tensor("out", [ROWS_PER_CORE, 7, W_IMG], mybir.dt.float32,
                         kind="ExternalOutput")

    with tile.TileContext(nc) as tc:
        with (
            tc.tile_pool(name="consts", bufs=1) as consts,
            tc.tile_pool(name="alpha", bufs=4) as apool,
            tc.tile_pool(name="sigma", bufs=3, space="PSUM") as spool,
            tc.tile_pool(name="outp", bufs=3, space="PSUM") as opool,
        ):
            w1_sb = consts.tile([3 * n_chunks, ROWS_PER_CORE * NSEG * 128],
                                mybir.dt.float32)
            nc.sync.dma_start(w1_sb[:], w1.ap()[:])
            c2_sb = consts.tile([128, ROWS_PER_CORE * n_chunks * 7],
                                mybir.dt.float16)
            nc.sync.dma_start(c2_sb[:], c2.ap()[:])
            p3_sb = consts.tile([3, W_IMG], mybir.dt.float32)
            nc.sync.dma_start(p3_sb[:], p3.ap()[:])

            import contextlib
            rep_ctx = (tc.For_i(0, reps, 1) if reps > 1
                       else contextlib.nullcontext())
            with rep_ctx:
                _body(nc, tc, n_chunks, w1_sb, c2_sb, p3_sb, out,
                      apool, spool, opool)

    nc.compile()
    _compiled[(n_chunks, reps)] = nc
    return nc


def _body(nc, tc, n_chunks, w1_sb, c2_sb, p3_sb, out, apool, spool, opool):
    from concourse import mybir
    if True:
        if True:
            for r in range(ROWS_PER_CORE):
                out_ps = opool.tile([7, W_IMG], mybir.dt.float32)
                for ch in range(n_chunks):
                    sig = spool.tile([128, W_IMG], mybir.dt.float32)
                    for s in range(NSEG):
                        wcol = (r * NSEG + s) * 128
                        nc.tensor.matmul(
                            sig[:, s * SEGW:(s + 1) * SEGW],
                            w1_sb[3 * ch:3 * ch + 3, wcol:wcol + 128],
                            p3_sb[:, s * SEGW:(s + 1) * SEGW],
                            start=True, stop=True,
                        )
                    alpha = apool.tile([128, W_IMG], mybir.dt.float16,
                                       tag="alpha")
                    nc.scalar.activation(alpha[:], sig[:],
                                         mybir.ActivationFunctionType.Exp,
                                         scale=-1.0)
                    alpham = apool.tile([128, W_IMG], mybir.dt.float16,
                                        tag="alpham")
                    nc.vector.scalar_tensor_tensor(
                        alpham[:], alpha[:], 1.0 / 255.0, alpha[:],
                        mybir.AluOpType.is_ge, mybir.AluOpType.mult)
                    ccol = (r * n_chunks + ch) * 7
                    nc.tensor.matmul(
                        out_ps[:], c2_sb[:, ccol:ccol + 7], alpham[:],
                        start=(ch == 0), stop=(ch == n_chunks - 1),
                    )
                out_sb = apool.tile([7, W_IMG], mybir.dt.float32,
                                    tag="outsb")
                nc.vector.tensor_copy(out_sb[:], out_ps[:])
                nc.sync.dma_start(out.ap()[r], out_sb[:])


def _prepare(_xyz, _cholesky, _features_dc, random_colors):
    """Host-side projection, per-row culling and slab gather (f64 coeffs)."""
    xyz = np.asarray(_xyz, np.float32)
    chol = np.asarray(_cholesky, np.float32)
    means = np.tanh(xyz)
    L = chol + np.array([0.5, 0.0, 0.5], np.float32)
    l0 = L[:, 0].astype(np.float64)
    l1 = L[:, 1].astype(np.float64)
    l2 = L[:, 2].astype(np.float64)
    s00, s01, s11 = l0 * l0, l0 * l1, l1 * l1 + l2 * l2
    det = s00 * s11 - s01 * s01
    a, b, c = s11 / det, -s01 / det, s00 / det
    cx = 0.5 * ((means[:, 0].astype(np.float64) + 1) * W_IMG - 1)
    cy = 0.5 * ((means[:, 1].astype(np.float64) + 1) * H_IMG - 1)

    qy = 0.5 * (c - b * b / a)
    ry = np.sqrt((SCUT + CULL_MARGIN) / qy)

    rows_idx = [np.nonzero(np.abs(cy - r) <= ry)[0] for r in range(H_IMG)]
    maxcount = max(len(i) for i in rows_idx)
    n_chunks = max(1, -(-maxcount // 128))

    colors = np.asarray(_features_dc, np.float32)
    geomc = np.asarray(random_colors, np.float32) * np.float32(0.5)

    # w1: [core][coeff(3) x chunk][row][seg][j]; c2: [core][j][row][chunk][7]
    w1_all = np.zeros((N_CORES, n_chunks, 3, ROWS_PER_CORE, NSEG, 128),
                      np.float32)
    w1_all[:, :, 2] = PAD_SIGMA
    c2_all = np.zeros((N_CORES, 128, ROWS_PER_CORE, n_chunks, 7), np.float16)

    xc = np.arange(NSEG) * SEGW + (SEGW - 1) / 2.0          # segment centers
    for r in range(H_IMG):
        idx = rows_idx[r]
        n = len(idx)
        if n == 0:
            continue
        core, rl = divmod(r, ROWS_PER_CORE)
        dy = cy[idx] - r
        aa, bb, cc_ = a[idx], b[idx], c[idx]
        cxp = cx[idx][None, :] - xc[:, None]                # [NSEG, n]
        w0 = np.broadcast_to(0.5 * aa, (NSEG, n))
        w1c = -(aa * cxp + (bb * dy)[None, :])
        w2c = 0.5 * aa * cxp * cxp + (bb * dy)[None, :] * cxp \
            + (0.5 * cc_ * dy * dy)[None, :]
        flat = np.stack([w0, w1c, w2c]).astype(np.float32)  # [3, NSEG, n]
        ch_i, j_i = np.divmod(np.arange(n), 128)
        w1_all[core, ch_i, :, rl, :, j_i] = flat.transpose(2, 0, 1)
        c2_all[core, j_i, rl, ch_i, 0:3] = colors[idx]
        c2_all[core, j_i, rl, ch_i, 3:6] = geomc[idx]
        c2_all[core, j_i, rl, ch_i, 6] = np.float16(1.0)

    pxs = np.arange(W_IMG, dtype=np.float64)
    pxl = pxs - xc[(pxs // SEGW).astype(int)]               # segment-local
    p3 = np.stack([pxl * pxl, pxl, np.ones(W_IMG)]).astype(np.float32)

    in_maps = [
        {
            "w1": np.ascontiguousarray(
                w1_all[i].reshape(3 * n_chunks, ROWS_PER_CORE * NSEG * 128)),
            "c2": np.ascontiguousarray(
                c2_all[i].reshape(128, ROWS_PER_CORE * n_chunks * 7)),
            "p3": p3,
        }
        for i in range(N_CORES)
    ]
    return in_maps, n_chunks


def kernel(_xyz, _cholesky, _features_dc, random_colors, H, W):
    assert int(H) == H_IMG and int(W) == W_IMG, (H, W)
    from concourse.bass_utils import run_bass_kernel_spmd

    in_maps, n_chunks = _prepare(_xyz, _cholesky, _features_dc, random_colors)
    nc = _build(n_chunks)
    res = run_bass_kernel_spmd(nc, in_maps, core_ids=list(range(N_CORES)),
                               trace=False)
    acc = np.concatenate([res.results[i]["out"] for i in range(N_CORES)],
                         axis=0)                            # [H, 7, W]
    render = np.clip(acc[:, 0:3, :], 0.0, 1.0).transpose(1, 0, 2)[None]
    gauss_render = np.clip(acc[:, 3:6, :], 0.0, 1.0).transpose(1, 0, 2)[None]
    alpha_map = acc[:, 6, :][None, None]
    opac = np.ones((np.asarray(_xyz).shape[0], 1), np.float32)
    return (np.ascontiguousarray(render.astype(np.float32)),
            np.ascontiguousarray(gauss_render.astype(np.float32)),
            np.ascontiguousarray(alpha_map.astype(np.float32)),
            opac)


# revision 9
# speedup vs baseline: 2.3283x; 2.3283x over previous
"""GaussianImage_Cholesky rasterizer on 8 Trainium2 NeuronCores.

Strategy: pixel-parallel across cores (core i owns image rows 64i..64i+63).
Host culls gaussians per image row (pixels further than ry in y can never
reach alpha >= 1/255, which the reference zeroes), gathers per-row slabs
padded to a multiple of 128, and precomputes per-(row, segment) quadratic
coefficients so the device evaluates

    sigma[g, px] = w0_g*px_s^2 + w1_g*px_s + w2_g        (px_s segment-local)

as a K=3 fp32 matmul. Per row on device:
    4x matmul fp32  : sigma psum[128,512]  (one per 128-px segment)
    ACT Exp         : alpha = exp(-sigma) -> fp16 sbuf
    DVE (fused)     : alpha = (alpha >= 1/255) * alpha
    matmul fp16     : out psum[7,512] = C[128,7].T @ alpha  (rgb,geom,ones)
    DMA             : psum -> dram out[row]
Host reassembles rows, clips, and emits (render, gauss_render, alpha_map,
opac).
"""
import sys
import numpy as np

if "/opt/trn_rl_repo" not in sys.path:
    sys.path.insert(0, "/opt/trn_rl_repo")

N_CORES = 8
H_IMG = 512
W_IMG = 512
ROWS_PER_CORE = H_IMG // N_CORES
NSEG = 4
SEGW = W_IMG // NSEG
SCUT = float(np.log(255.0))
CULL_MARGIN = 0.03
PAD_SIGMA = 100.0

_compiled = {}


def _build(n_chunks, reps=1):
    """Compile the SPMD kernel for a given number of 128-gaussian chunks/row.

    reps>1 wraps the row loop in a hardware For_i that re-executes the whole
    body `reps` times — used only for marginal-cost timing in test.py.
    """
    if (n_chunks, reps) in _compiled:
        return _compiled[(n_chunks, reps)]

    import concourse.tile as tile
    from concourse import bacc, mybir

    nc = bacc.Bacc("TRN2", target_bir_lowering=False, debug=False,
                   num_devices=N_CORES)

    w1 = nc.dram_tensor("w1", [48, ROWS_PER_CORE * n_chunks * 128],
                        mybir.dt.bfloat16, kind="ExternalInput")
    c2 = nc.dram_tensor("c2", [128, ROWS_PER_CORE * n_chunks * 7],
                        mybir.dt.float16, kind="ExternalInput")
    p3 = nc.dram_tensor("p3", [48, W_IMG], mybir.dt.bfloat16,
                        kind="ExternalInput")
    out = nc.dram_tensor("out", [ROWS_PER_CORE // 16, 128, 448],
                         mybir.dt.float32, kind="ExternalOutput")

    with tile.TileContext(nc) as tc:
        with (
            tc.tile_pool(name="consts", bufs=1) as consts,
            tc.tile_pool(name="alpha", bufs=4) as apool,
            tc.tile_pool(name="sigma", bufs=3, space="PSUM") as spool,
            tc.tile_pool(name="outp", bufs=3, space="PSUM") as opool,
        ):
            w1_sb = consts.tile([48, ROWS_PER_CORE * n_chunks * 128],
                                mybir.dt.bfloat16)
            nc.sync.dma_start(w1_sb[:], w1.ap()[:])
            c2_sb = consts.tile([128, ROWS_PER_CORE * n_chunks * 7],
                                mybir.dt.float16)
            nc.sync.dma_start(c2_sb[:], c2.ap()[:])
            p3_sb = consts.tile([48, W_IMG], mybir.dt.bfloat16)
            nc.sync.dma_start(p3_sb[:], p3.ap()[:])

            import contextlib
            rep_ctx = (tc.For_i(0, reps, 1) if reps > 1
                       else contextlib.nullcontext())
            with rep_ctx:
                _body(nc, tc, n_chunks, w1_sb, c2_sb, p3_sb, out,
                      apool, spool, opool)

    nc.compile()
    _compiled[(n_chunks, reps)] = nc
    return nc


def _body(nc, tc, n_chunks, w1_sb, c2_sb, p3_sb, out, apool, spool, opool):
    """Software-pipelined emission: per tick t, PE sees mm1(t) before mm2(t-2),
    so sigma for future rows streams while ACT/DVE finish previous rows."""
    from concourse import mybir
    R = ROWS_PER_CORE
    alpham = {}
    sig = {}
    out_ps = {}
    for t in range(R + 2):
        if t < R:
            r = t
            sig[r] = []
            for ch in range(n_chunks):
                sg = spool.tile([128, W_IMG], mybir.dt.float32, tag="sig")
                wcol = (r * n_chunks + ch) * 128
                nc.tensor.matmul(sg[:], w1_sb[:, wcol:wcol + 128], p3_sb[:],
                                 start=True, stop=True)
                sig[r].append(sg)
        if 0 <= t - 1 < R:
            r = t - 1
            alpham[r] = []
            for ch in range(n_chunks):
                alpha = apool.tile([128, W_IMG], mybir.dt.float16, tag="alpha")
                nc.scalar.activation(alpha[:], sig[r][ch][:],
                                     mybir.ActivationFunctionType.Exp,
                                     scale=-1.0)
                am = apool.tile([128, W_IMG], mybir.dt.float16, tag="alpham")
                nc.vector.scalar_tensor_tensor(
                    am[:], alpha[:], 1.0 / 255.0, alpha[:],
                    mybir.AluOpType.is_ge, mybir.AluOpType.mult)
                alpham[r].append(am)
            del sig[r]
        if 0 <= t - 2 < R:
            r = t - 2
            g, r16 = divmod(r, 16)
            if r16 == 0:
                out_ps[g] = opool.tile([128, 448], mybir.dt.float32, tag="outps", name="outps")
            for ch in range(n_chunks):
                ccol = (r * n_chunks + ch) * 7
                for q in range(4):
                    nc.tensor.matmul(
                        out_ps[g][:, 28 * r16 + 7 * q:28 * r16 + 7 * q + 7],
                        alpham[r][ch][:, 128 * q:128 * q + 128],
                        c2_sb[:, ccol:ccol + 7],
                        start=(ch == 0), stop=(ch == n_chunks - 1),
                    )
            del alpham[r]
            if r16 == 15:
                out_sb = apool.tile([128, 448], mybir.dt.float32, tag="outsb")
                nc.vector.tensor_copy(out_sb[:], out_ps[g][:])
                nc.sync.dma_start(out.ap()[g], out_sb[:])
                del out_ps[g]


def _prepare(_xyz, _cholesky, _features_dc, random_colors):
    """Host-side projection, per-row culling, slab gather, bf16 weight split."""
    import ml_dtypes
    bf = ml_dtypes.bfloat16
    xyz = np.asarray(_xyz, np.float32)
    chol = np.asarray(_cholesky, np.float32)
    means = np.tanh(xyz)
    L = chol + np.array([0.5, 0.0, 0.5], np.float32)
    l0 = L[:, 0].astype(np.float64)
    l1 = L[:, 1].astype(np.float64)
    l2 = L[:, 2].astype(np.float64)
    s00, s01, s11 = l0 * l0, l0 * l1, l1 * l1 + l2 * l2
    det = s00 * s11 - s01 * s01
    a, b, c = s11 / det, -s01 / det, s00 / det
    cx = 0.5 * ((means[:, 0].astype(np.float64) + 1) * W_IMG - 1)
    cy = 0.5 * ((means[:, 1].astype(np.float64) + 1) * H_IMG - 1)

    qy = 0.5 * (c - b * b / a)
    ry = np.sqrt((SCUT + CULL_MARGIN) / qy)

    rows_idx = [np.nonzero(np.abs(cy - r) <= ry)[0] for r in range(H_IMG)]
    maxcount = max(len(i) for i in rows_idx)
    n_chunks = max(1, -(-maxcount // 128))

    colors = np.asarray(_features_dc, np.float32)
    geomc = np.asarray(random_colors, np.float32) * np.float32(0.5)

    def split3(w):
        t1 = w.astype(bf).astype(np.float64)
        r1 = w - t1
        t2 = r1.astype(bf).astype(np.float64)
        t3 = (r1 - t2)
        return t1, t2, t3

    # w48: [core][12*seg+k][row][chunk][j]  (bf16)
    w1_all = np.zeros((N_CORES, 48, ROWS_PER_CORE, n_chunks, 128), np.float64)
    w1_all[:, 9::12] = PAD_SIGMA      # w2 term-1 rows: pad sigma=100
    c2_all = np.zeros((N_CORES, 128, ROWS_PER_CORE, n_chunks, 7), np.float16)

    for r in range(H_IMG):
        idx = rows_idx[r]
        n = len(idx)
        if n == 0:
            continue
        core, rl = divmod(r, ROWS_PER_CORE)
        dy = cy[idx] - r
        aa, bb, cc_ = a[idx], b[idx], c[idx]
        ch_i, j_i = np.divmod(np.arange(n), 128)
        for s in range(NSEG):
            x0 = SEGW * s
            cxp = cx[idx] - x0
            w0 = 0.5 * aa
            w1c = -(aa * cxp + bb * dy)
            w2c = 0.5 * aa * cxp * cxp + bb * cxp * dy + 0.5 * cc_ * dy * dy
            w0a, w0b, w0c = split3(w0)
            w1a, w1b, w1cc = split3(w1c)
            w2a, w2b, w2cc = split3(w2c)
            Wm = np.stack([w0a, w0a, w0b, w0b, w0c, w0c,
                           w1a, w1b, w1cc, w2a, w2b, w2cc])   # [12, n]
            w1_all[core, 12 * s:12 * s + 12, rl, ch_i, j_i] = Wm.T
        c2_all[core, j_i, rl, ch_i, 0:3] = colors[idx]
        c2_all[core, j_i, rl, ch_i, 3:6] = geomc[idx]
        c2_all[core, j_i, rl, ch_i, 6] = np.float16(1.0)

    # features [48, 512]: block s rows 12s..12s+12 active in cols 128s..+128
    pxl = np.arange(SEGW, dtype=np.float64)
    p2 = pxl * pxl
    p2h = p2.astype(bf).astype(np.float64)
    p2l = p2 - p2h
    FEAT = np.stack([p2h, p2l, p2h, p2l, p2h, p2l, pxl, pxl, pxl,
                     np.ones(SEGW), np.ones(SEGW), np.ones(SEGW)])
    p48 = np.zeros((48, W_IMG), np.float64)
    for s in range(NSEG):
        p48[12 * s:12 * s + 12, SEGW * s:SEGW * (s + 1)] = FEAT

    in_maps = [
        {
            "w1": np.ascontiguousarray(w1_all[i].reshape(
                48, ROWS_PER_CORE * n_chunks * 128)).astype(bf),
            "c2": np.ascontiguousarray(
                c2_all[i].reshape(128, ROWS_PER_CORE * n_chunks * 7)),
            "p3": p48.astype(bf),
        }
        for i in range(N_CORES)
    ]
    return in_maps, n_chunks


def kernel(_xyz, _cholesky, _features_dc, random_colors, H, W):
    assert int(H) == H_IMG and int(W) == W_IMG, (H, W)
    from concourse.bass_utils import run_bass_kernel_spmd

    in_maps, n_chunks = _prepare(_xyz, _cholesky, _features_dc, random_colors)
    nc = _build(n_chunks)
    res = run_bass_kernel_spmd(nc, in_maps, core_ids=list(range(N_CORES)),
                               trace=False)
    # per core: out [4, 128, 448] -> [64, 7, 512]
    percore = []
    for i in range(N_CORES):
        o = res.results[i]["out"].reshape(4, 128, 16, 4, 7)
        percore.append(o.transpose(0, 2, 4, 3, 1).reshape(64, 7, 512))
    acc = np.concatenate(percore, axis=0)                   # [H, 7, W]
    render = np.clip(acc[:, 0:3, :], 0.0, 1.0).transpose(1, 0, 2)[None]
    gauss_render = np.clip(acc[:, 3:6, :], 0.0, 1.0).transpose(1, 0, 2)[None]
    alpha_map = acc[:, 6, :][None, None]
    opac = np.ones((np.asarray(_xyz).shape[0], 1), np.float32)
    return (np.ascontiguousarray(render.astype(np.float32)),
            np.ascontiguousarray(gauss_render.astype(np.float32)),
            np.ascontiguousarray(alpha_map.astype(np.float32)),
            opac)
